# revision 1
# baseline (speedup 1.0000x reference)
"""Trainium2 Bass kernel for 16-head causal MultiHeadAttention.

Problem: N=4, T=2048, D_M=1024, HEADS=16, D_K=64, fp32, causal + key pad mask.

Sharding (8 cores): core c handles batch n = c//2 and head group g = c%2
(8 heads each).  Each core computes its batch's projections restricted to its
8 heads, causal attention for those heads, and a partial output projection
(A_heads @ Wo_rows).  The host sums the two partials per batch.

Device dataflow (transpose-free):
  - Host passes q/k/v pre-transposed (xT: [D_M, T]) so the projection
    contraction dim (d) lands on SBUF partitions.
  - qhT/khT ([j, T], head dim on partitions) come straight from the
    projection matmuls; vh ([T, dv]) likewise.
  - Scores are computed transposed, S^T[tk, tq] = khT^T-slice @ qhT-slice,
    so softmax's exp is elementwise from PSUM and attn@V consumes exp(S^T)
    directly: A^T[dv, tq] = vh^T @ exp(S^T) with vh in natural layout.
  - A ones-column appended to vh makes row 64 of the attn@V accumulator the
    softmax denominator (sum_tk exp) for free.
  - A_norm^T is exactly the lhsT the output projection needs; out[tq, e]
    comes out in natural layout for the store.
All matmuls run as float32r (full PE rate, fp32 storage).
"""

import os
import sys

import numpy as np

for _p in ("/opt/trn_rl_repo",):
    if _p not in sys.path and os.path.isdir(_p):
        sys.path.insert(0, _p)

import concourse.bacc as bacc
import concourse.bass as bass
import concourse.mybir as mybir
import concourse.tile as tile
from concourse.bass_utils import run_bass_kernel_spmd

# Problem constants (hardcoded per harness contract).
D_M = 1024
HEADS = 16
D_K = 64
N_B = 4
T = 2048
N_CORES = 8
HPC = HEADS // 2          # heads per core = 8
J = HPC * D_K             # per-core projection width = 512
G = J // 128              # j-tiles per core = 4
DT = D_M // 128           # d (contraction) tiles = 8
CHUNK = 512               # tq chunk (free dim of most matmuls)
NCHUNK = T // CHUNK       # 4
NBLK = T // 128           # tk blocks = 16
F32 = mybir.dt.float32
F32R = mybir.dt.float32r
NEG = -1.0e30

_cached_nc = None


def r(ap):
    """float32r view (no-op for tiles already declared float32r)."""
    return ap if ap.dtype == F32R else ap.bitcast(F32R)


def build_nc(loop_n=None, phases=('kv', 'q', 'attn', 'out')):
    """loop_n: if set, wrap the compute body in a HW For_i loop (timing
    variant — K projection reuses the Q weights so weight slots load once;
    outputs are numerically wrong but the instruction stream is identical)."""
    import contextlib
    nc = bacc.Bacc(None)

    xq = nc.declare_dram_parameter("xq_t", [D_M, T], F32R, isOutput=False)
    xk = nc.declare_dram_parameter("xk_t", [D_M, T], F32R, isOutput=False)
    xv = nc.declare_dram_parameter("xv_t", [D_M, T], F32R, isOutput=False)
    wq = nc.declare_dram_parameter("wq", [D_M, J], F32R, isOutput=False)
    wk = nc.declare_dram_parameter("wk", [D_M, J], F32R, isOutput=False)
    wv = nc.declare_dram_parameter("wv", [D_M, J], F32R, isOutput=False)
    bq = nc.declare_dram_parameter("bq2", [128, G], F32, isOutput=False)
    bk = nc.declare_dram_parameter("bk2", [128, G], F32, isOutput=False)
    bvb = nc.declare_dram_parameter("bvb", [128, J], F32, isOutput=False)
    wo = nc.declare_dram_parameter("wo", [J, D_M], F32R, isOutput=False)
    bob = nc.declare_dram_parameter("bob", [128, D_M], F32, isOutput=False)
    padb = nc.declare_dram_parameter("padb", [128, NBLK], F32, isOutput=False)
    trim = nc.declare_dram_parameter("trimask", [128, 128], F32, isOutput=False)
    out = nc.declare_dram_parameter("out", [T, D_M], F32, isOutput=True)

    Ident = mybir.ActivationFunctionType.Identity
    Exp = mybir.ActivationFunctionType.Exp

    with tile.TileContext(nc) as tc:
        with (
            tc.tile_pool(name="consts", bufs=1) as cpool,
            tc.tile_pool(name="wproj", bufs=1) as wpool,
            tc.tile_pool(name="persist", bufs=1) as ppool,
            tc.tile_pool(name="xs", bufs=16) as xpool,
            tc.tile_pool(name="qht", bufs=6) as qpool,
            tc.tile_pool(name="ant", bufs=6) as apool,
            tc.tile_pool(name="exps", bufs=3) as epool,
            tc.tile_pool(name="rec", bufs=2) as rpool,
            tc.tile_pool(name="osb", bufs=3) as opool,
            tc.tile_pool(name="ps_x", bufs=3, space="PSUM") as psum_px,
            tc.tile_pool(name="ps_a", bufs=2, space="PSUM") as psum_ap,
        ):
            # ---- constants -------------------------------------------------
            trim_t = cpool.tile([128, 128], F32, name="trim_t", tag="trim")
            nc.sync.dma_start(out=trim_t[:, :], in_=trim[:, :])
            padb_t = cpool.tile([128, NBLK], F32, name="padb_t", tag="padb")
            nc.sync.dma_start(out=padb_t[:, :], in_=padb[:, :])
            bq_t = cpool.tile([128, G], F32, name="bq_t", tag="bq")
            nc.sync.dma_start(out=bq_t[:, :], in_=bq[:, :])
            bk_t = cpool.tile([128, G], F32, name="bk_t", tag="bk")
            nc.sync.dma_start(out=bk_t[:, :], in_=bk[:, :])
            bvb_t = cpool.tile([128, J], F32, name="bvb_t", tag="bvb")
            nc.sync.dma_start(out=bvb_t[:, :], in_=bvb[:, :])
            bob_t = cpool.tile([128, D_M], F32, name="bob_t", tag="bob")
            nc.sync.dma_start(out=bob_t[:, :], in_=bob[:, :])

            # Output-projection weights, resident: wo_t[g] = wo[128g:+128, :]
            wo_t = []
            for g in range(G):
                t_ = wpool.tile([128, D_M], F32R, name=f"wo_t{g}", tag=f"wo{g}")
                nc.sync.dma_start(out=t_[:, :], in_=wo[g * 128:(g + 1) * 128, :])
                wo_t.append(t_)

            # V-projection rhs weights, resident: wv_t[d] = wv[128d:+128, :]
            wv_t = []
            for d in range(DT):
                t_ = wpool.tile([128, J], F32R, name=f"wv_t{d}", tag=f"wv{d}")
                nc.sync.dma_start(out=t_[:, :], in_=wv[d * 128:(d + 1) * 128, :])
                wv_t.append(t_)

            # Q/K projection weights [128,128] tiles. wk first; wq reuses the
            # same slots after the K projection finishes (shared tags).
            def load_w(dram, label):
                ts_ = {}
                for d in range(DT):
                    for g in range(G):
                        t_ = wpool.tile(
                            [128, 128], F32R, name=f"{label}_{d}_{g}",
                            tag=f"wqk{d}_{g}", bufs=1,
                        )
                        nc.sync.dma_start(
                            out=t_[:, :],
                            in_=dram[d * 128:(d + 1) * 128, g * 128:(g + 1) * 128],
                        )
                        ts_[(d, g)] = t_
                return ts_

            wk_t = load_w(wk, "wk") if loop_n is None else None

            # Persistent activations.
            khT = []  # khT[g]: [128, T] — heads 2g (rows 0-63), 2g+1 (64-127)
            for g in range(G):
                t_ = ppool.tile([128, T], F32R, name=f"khT{g}", tag=f"khT{g}")
                khT.append(t_)
            vh = []   # vh[i]: [128, 520] — per head h: cols 65h..65h+63 = v, 65h+64 = 1
            for i in range(NBLK):
                t_ = ppool.tile([128, 520], F32R, name=f"vh{i}", tag=f"vh{i}")
                vh.append(t_)

            def load_x_chunk(xdram, c, label):
                tiles = []
                for d in range(DT):
                    t_ = xpool.tile([128, CHUNK], F32R, name=f"{label}{c}_{d}", tag="xs")
                    nc.sync.dma_start(
                        out=t_[:, :],
                        in_=xdram[d * 128:(d + 1) * 128, c * CHUNK:(c + 1) * CHUNK],
                    )
                    tiles.append(t_)
                return tiles

            def body(wk_t, wq_t, phases=phases):
                # ---- K projection -----------------------------------------
                for c in range(NCHUNK if 'kv' in phases else 0):
                    xk_c = load_x_chunk(xk, c, "xk")
                    for g in range(G):
                        ps = psum_px.tile([128, CHUNK], F32, name=f"psK{c}_{g}", tag="px", padded_shape=[128, 2 * CHUNK])
                        for d in range(DT):
                            nc.tensor.matmul(
                                ps[:, :], r(wk_t[(d, g)][:, :]), r(xk_c[d][:, :]),
                                start=(d == 0), stop=(d == DT - 1),
                            )
                        nc.vector.tensor_scalar_add(
                            khT[g][:, c * CHUNK:(c + 1) * CHUNK], ps[:, :],
                            bk_t[:, g:g + 1],
                        )

                # ---- V projection ---------------------------------------------
                for c in range(NCHUNK if 'kv' in phases else 0):
                    xv_c = load_x_chunk(xv, c, "xv")
                    for tl in range(4):
                        i = 4 * c + tl
                        ps = psum_px.tile([128, J], F32, name=f"psV{i}", tag="px", padded_shape=[128, 2 * CHUNK])
                        for d in range(DT):
                            nc.tensor.matmul(
                                ps[:, :],
                                r(xv_c[d][:, tl * 128:(tl + 1) * 128]),
                                r(wv_t[d][:, :]),
                                start=(d == 0), stop=(d == DT - 1),
                            )
                        # vh[i][:, 65h + (0..63)] = psum + bv ; vh[i][:, 65h+64] = 1
                        dst = vh[i][:, 0:J + HPC].rearrange("p (h e) -> p h e", e=65)
                        nc.vector.tensor_add(
                            dst[:, :, 0:64],
                            ps[:, :].rearrange("p (h e) -> p h e", e=64),
                            bvb_t[:, :].rearrange("p (h e) -> p h e", e=64),
                        )
                        nc.vector.memset(dst[:, :, 64:65].bitcast(F32), 1.0)

                if 'kv' not in phases and 'attn' in phases:
                    for g in range(G):
                        nc.vector.memset(khT[g][:, :].bitcast(F32), 0.0)
                    for i in range(NBLK):
                        nc.vector.memset(vh[i][:, :].bitcast(F32), 0.0)

                # ---- Q weights (reuse wk slots) --------------------------------
                if loop_n is None:
                    wq_t = load_w(wq, "wq")

                # ---- per-chunk: Q proj -> attention -> out proj ----------------
                for c in range(NCHUNK):
                    qht = []
                    if 'q' in phases:
                        xq_c = load_x_chunk(xq, c, "xq")
                    for g in range(G):
                        qt = qpool.tile([128, CHUNK], F32R, name=f"qht{c}_{g}", tag="qht")
                        if 'q' not in phases and 'attn' in phases:
                            nc.vector.memset(qt[:, :].bitcast(F32), 0.0)
                        if 'q' in phases:
                            ps = psum_px.tile([128, CHUNK], F32, name=f"psQ{c}_{g}", tag="px", padded_shape=[128, 2 * CHUNK])
                            for d in range(DT):
                                nc.tensor.matmul(
                                    ps[:, :], r(wq_t[(d, g)][:, :]), r(xq_c[d][:, :]),
                                    start=(d == 0), stop=(d == DT - 1),
                                )
                            nc.vector.tensor_scalar_add(
                                qt[:, :], ps[:, :], bq_t[:, g:g + 1],
                            )
                        qht.append(qt)

                    nb = 4 * c + 4  # causal: tk blocks 0..nb-1
                    ant = []
                    for g in range(G):
                        at = apool.tile([128, CHUNK], F32R, name=f"ant{c}_{g}", tag="ant")
                        ant.append(at)
                        if 'attn' not in phases and 'out' in phases:
                            nc.vector.memset(at[:, :].bitcast(F32), 0.0)
                        if 'attn' not in phases:
                            continue
                        ps_a = [psum_ap.tile([65, CHUNK], F32,
                                             name=f"psA{c}_{2 * g + hh}", tag="pa")
                                for hh in range(2)]
                        # 2-deep software pipeline: A(bk-2) is emitted after
                        # S(bk), so each A pair waits on an exp that had a
                        # full block of ACT time to finish.  Both heads of a
                        # block share one [128,1024] PSUM pair-tile (2 banks)
                        # and one [128,1024] exp tile -> a single ACT op per
                        # block pair.
                        pend = []  # [(cs, es_pair), ...] oldest first
                        def emit_a(bk_, first):
                            pcs, pes = pend.pop(0)
                            for hh in range(2):
                                nc.tensor.matmul(
                                    ps_a[hh][:, pcs:],
                                    r(vh[bk_][:, 65 * (2 * g + hh):65 * (2 * g + hh) + 65]),
                                    r(pes[:, 512 * hh + pcs:512 * hh + 512]),
                                    start=first, stop=(bk_ == nb - 1),
                                )
                        for bk in range(nb):
                            m = bk - 4 * c  # >=0 on the diagonal superblock
                            cs = min(128 * m, 256) if m >= 0 else 0  # compute start
                            ms = 128 * m if m >= 0 else 0            # causal start
                            ps_s = psum_px.tile([128, 2 * CHUNK], F32,
                                                name=f"psS{c}_{g}_{bk}", tag="px")
                            for hh in range(2):
                                nc.tensor.matmul(
                                    ps_s[:, 512 * hh + cs:512 * hh + 512],
                                    r(khT[g][hh * 64:(hh + 1) * 64, bk * 128:(bk + 1) * 128]),
                                    r(qht[g][hh * 64:(hh + 1) * 64, cs:]),
                                    start=True, stop=True,
                                )
                            if m >= 0:
                                # triangular additive mask on both heads' diag blocks
                                for hh in range(2):
                                    nc.vector.tensor_add(
                                        ps_s[:, 512 * hh + ms:512 * hh + ms + 128],
                                        ps_s[:, 512 * hh + ms:512 * hh + ms + 128],
                                        trim_t[:, :],
                                    )
                            if bk >= 2:
                                emit_a(bk - 2, first=(bk == 2))
                            es = epool.tile([128, 2 * CHUNK], F32R,
                                            name=f"es{c}_{g}_{bk}", tag="es")
                            if ms > 0:
                                # one strided ACT op covering [ms:512] of both halves
                                nc.scalar.activation(
                                    es[:, 0:2 * CHUNK].rearrange(
                                        "p (h e) -> p h e", h=2)[:, :, ms:],
                                    ps_s[:, 0:2 * CHUNK].rearrange(
                                        "p (h e) -> p h e", h=2)[:, :, ms:],
                                    Exp, bias=padb_t[:, bk:bk + 1], scale=0.125,
                                )
                                if ms > cs:
                                    for hh in range(2):
                                        nc.vector.memset(
                                            es[:, 512 * hh + cs:512 * hh + ms].bitcast(F32), 0.0)
                            else:
                                nc.scalar.activation(
                                    es[:, :], ps_s[:, :], Exp,
                                    bias=padb_t[:, bk:bk + 1], scale=0.125,
                                )
                            pend.append((cs, es))
                        emit_a(nb - 2, first=False)
                        emit_a(nb - 1, first=False)
                        for hh in range(2):
                            h = 2 * g + hh
                            # normalize: rows 0-63 = A^T numerator, row 64 = denom.
                            # NB: partition_broadcast reads partition 0 of the
                            # underlying tile regardless of the input AP's
                            # partition offset, so the reciprocal must land on
                            # partition 0 (cross-base DVE write is fine on HW).
                            rc = rpool.tile([128, CHUNK], F32, name=f"rc{c}_{h}", tag="rc")
                            nc.vector.reciprocal(rc[0:1, :], ps_a[hh][64:65, :])
                            rb = rpool.tile([128, CHUNK], F32, name=f"rb{c}_{h}", tag="rb")
                            nc.gpsimd.partition_broadcast(rb[0:64, :], rc[0:1, :])
                            nc.vector.tensor_mul(
                                at[hh * 64:(hh + 1) * 64, :], ps_a[hh][0:64, :], rb[0:64, :],
                            )

                    # out[tq, e] = sum_g ant[g][:, tq-tile].T @ wo_t[g] + bo
                    for mt in range(4 if 'out' in phases else 0):
                        row0 = (4 * c + mt) * 128
                        for e in range(2):
                            ps = psum_px.tile([128, CHUNK], F32,
                                              name=f"psO{c}_{mt}_{e}", tag="px",
                                              padded_shape=[128, 2 * CHUNK])
                            for g in range(G):
                                nc.tensor.matmul(
                                    ps[:, :],
                                    r(ant[g][:, mt * 128:(mt + 1) * 128]),
                                    r(wo_t[g][:, e * CHUNK:(e + 1) * CHUNK]),
                                    start=(g == 0), stop=(g == G - 1),
                                )
                            ob = opool.tile([128, CHUNK], F32, name=f"ob{c}_{mt}_{e}", tag="ob")
                            nc.vector.tensor_add(ob[:, :], ps[:, :],
                                                 bob_t[:, e * CHUNK:(e + 1) * CHUNK])
                            nc.sync.dma_start(
                                out=out[row0:row0 + 128, e * CHUNK:(e + 1) * CHUNK],
                                in_=ob[:, :],
                            )

            if loop_n is not None:
                wq_t = load_w(wq, "wq")
                with tc.For_i(0, loop_n, 1):
                    body(wq_t, wq_t)
            else:
                body(wk_t, None)

    nc.finalize()
    return nc


def get_nc():
    global _cached_nc
    if _cached_nc is None:
        _cached_nc = build_nc()
    return _cached_nc


def make_in_maps(q, k, v, pad_mask, Wq, bq, Wk, bk, Wv, bv, Wo, bo):
    """Host-side sharding: core c -> batch c//2, head-group c%2."""
    f = np.float32
    tri = np.where(
        np.arange(128)[None, :] >= np.arange(128)[:, None], 0.0, NEG
    ).astype(f)  # [tk, tq]: allow tq >= tk
    in_maps = []
    xT = {}
    for n in range(N_B):
        xT[n] = (
            np.ascontiguousarray(np.asarray(q[n], f).T),
            np.ascontiguousarray(np.asarray(k[n], f).T),
            np.ascontiguousarray(np.asarray(v[n], f).T),
        )
    for c in range(N_CORES):
        n, grp = divmod(c, 2)
        js = slice(grp * J, (grp + 1) * J)
        pb = np.where(np.asarray(pad_mask[n]) == 0, NEG, 0.0).astype(f)
        in_maps.append({
            "xq_t": xT[n][0], "xk_t": xT[n][1], "xv_t": xT[n][2],
            "wq": np.ascontiguousarray(np.asarray(Wq, f)[:, js]),
            "wk": np.ascontiguousarray(np.asarray(Wk, f)[:, js]),
            "wv": np.ascontiguousarray(np.asarray(Wv, f)[:, js]),
            "bq2": np.ascontiguousarray(np.asarray(bq, f)[js].reshape(G, 128).T),
            "bk2": np.ascontiguousarray(np.asarray(bk, f)[js].reshape(G, 128).T),
            "bvb": np.broadcast_to(np.asarray(bv, f)[js], (128, J)).copy(),
            "wo": np.ascontiguousarray(np.asarray(Wo, f)[js, :]),
            "bob": (np.broadcast_to(np.asarray(bo, f), (128, D_M)).copy()
                    if grp == 0 else np.zeros((128, D_M), f)),
            "padb": np.ascontiguousarray(pb.reshape(NBLK, 128).T),
            "trimask": tri,
        })
    return in_maps


def kernel(**inputs) -> np.ndarray:
    nc = get_nc()
    in_maps = make_in_maps(**inputs)
    res = run_bass_kernel_spmd(nc, in_maps, list(range(N_CORES))).results
    out = np.empty((N_B, T, D_M), np.float32)
    for n in range(N_B):
        out[n] = res[2 * n]["out"] + res[2 * n + 1]["out"]
    return out



# revision 2
# speedup vs baseline: 1.2175x; 1.2175x over previous
"""Trainium2 Bass kernel for 16-head causal MultiHeadAttention (v2).

Problem: N=4, T=2048, D_M=1024, HEADS=16, D_K=64, fp32, causal + key pad mask.

Sharding (8 cores): core c handles batch n = c//2 and head group g = c%2
(8 heads each).  Each core computes its batch's projections restricted to its
8 heads, causal attention for those heads, and a partial output projection
(A_heads @ Wo_rows).  The host sums the two partials per batch.

v2 changes vs v1:
  - x / khT / vh / qht / es / at tiles in bf16 (inputs + attention operands).
    All matmul operands are bf16 (the compiler rejects mixed bf16/fp32r
    matmuls); PSUM accumulation stays fp32.  Measured end-to-end rel err
    ~5e-3, well under the 2e-2 gate.  Halves x DMA traffic and SBUF use, and
    lifts the fp32r free>=256 matmul constraint so causal diag blocks compute
    exactly the needed columns.
  - wq and wk live in separate SBUF slots (no mid-kernel weight reload).
  - Chunk-level software pipeline: projections of chunk c+1 and the output
    projection of chunk c-1 are emitted as PE "filler" interleaved between
    attention blocks of chunk c, so the Tensor engine never waits for the
    exp (ACT) stream.  PSUM budget: 2x[128,1024] score tiles + 2x[65,512]
    attn accumulators + 2x[128,512] projection tiles = 8 banks exactly.

Device dataflow (transpose-free), as v1:
  - Host passes q/k/v pre-transposed (xT: [D_M, T], bf16) so the projection
    contraction dim lands on SBUF partitions.
  - Scores computed transposed S^T[tk, tq]; softmax exp is elementwise from
    PSUM; attn@V consumes exp(S^T) directly; a ones-column in vh makes row 64
    of the attn@V accumulator the softmax denominator.
  - A_norm^T is the lhsT the output projection needs; out[tq, e] lands in
    natural layout.
"""

import os
import sys

import numpy as np

for _p in ("/opt/trn_rl_repo",):
    if _p not in sys.path and os.path.isdir(_p):
        sys.path.insert(0, _p)

import concourse.bacc as bacc
import concourse.bass as bass
import concourse.mybir as mybir
import concourse.tile as tile
from concourse.bass_utils import run_bass_kernel_spmd

# Problem constants (hardcoded per harness contract).
D_M = 1024
HEADS = 16
D_K = 64
N_B = 4
T = 2048
N_CORES = 8
HPC = HEADS // 2          # heads per core = 8
J = HPC * D_K             # per-core projection width = 512
G = J // 128              # j-tiles per core = 4
DT = D_M // 128           # d (contraction) tiles = 8
CHUNK = 512               # tq chunk (free dim of most matmuls)
NCHUNK = T // CHUNK       # 4
NBLK = T // 128           # tk blocks = 16
F32 = mybir.dt.float32
F32R = mybir.dt.float32r
BF16 = mybir.dt.bfloat16
NEG = -1.0e30

_cached_nc = None


def r(ap):
    """matmul-operand view: f32 tiles as float32r, bf16 passes through."""
    if ap.dtype in (F32R, BF16):
        return ap
    return ap.bitcast(F32R)


def build_nc(loop_n=None, reps=1):
    """loop_n: wrap the body in a HW For_i loop (used for loop-slope timing).
    reps: emit the body N times sequentially (sim-only steady-state probe)."""
    nc = bacc.Bacc(None)

    xq = nc.declare_dram_parameter("xq_t", [D_M, T], BF16, isOutput=False)
    xk = nc.declare_dram_parameter("xk_t", [D_M, T], BF16, isOutput=False)
    xv = nc.declare_dram_parameter("xv_t", [D_M, T], BF16, isOutput=False)
    wq = nc.declare_dram_parameter("wq", [D_M, J], BF16, isOutput=False)
    wk = nc.declare_dram_parameter("wk", [D_M, J], BF16, isOutput=False)
    wv = nc.declare_dram_parameter("wv", [D_M, J], BF16, isOutput=False)
    bq = nc.declare_dram_parameter("bq2", [128, G], F32, isOutput=False)
    bk = nc.declare_dram_parameter("bk2", [128, G], F32, isOutput=False)
    bvb = nc.declare_dram_parameter("bvb", [128, J], F32, isOutput=False)
    wo = nc.declare_dram_parameter("wo", [J, D_M], BF16, isOutput=False)
    bob = nc.declare_dram_parameter("bob", [128, D_M], F32, isOutput=False)
    padb = nc.declare_dram_parameter("padb", [128, NBLK], F32, isOutput=False)
    trim = nc.declare_dram_parameter("trimask", [128, 128], F32, isOutput=False)
    out = nc.declare_dram_parameter("out", [T, D_M], F32, isOutput=True)

    Exp = mybir.ActivationFunctionType.Exp

    with tile.TileContext(nc) as tc:
        with (
            tc.tile_pool(name="consts", bufs=1) as cpool,
            tc.tile_pool(name="wproj", bufs=1) as wpool,
            tc.tile_pool(name="persist", bufs=1) as ppool,
            tc.tile_pool(name="xs", bufs=32) as xpool,
            tc.tile_pool(name="qht", bufs=8) as qpool,
            tc.tile_pool(name="ant", bufs=12) as apool,
            tc.tile_pool(name="exps", bufs=4) as epool,
            tc.tile_pool(name="rec", bufs=2) as rpool,
            tc.tile_pool(name="osb", bufs=3) as opool,
            tc.tile_pool(name="ps_s", bufs=2, space="PSUM") as psum_s,
            tc.tile_pool(name="ps_a", bufs=2, space="PSUM") as psum_a,
            tc.tile_pool(name="ps_p", bufs=2, space="PSUM") as psum_p,
        ):
            # ---- persistent weight/const loads (DMA order = consumption order)
            def load_w_tiles(dram, label):
                """Load [128,128] weight tiles as [128,256] g-pairs: 512-byte
                contiguous runs keep the DMA at full rate (bf16 128-col tiles
                would halve it)."""
                ts_ = {}
                for gp in range(G // 2):
                    for d in range(DT):
                        t_ = wpool.tile(
                            [128, 256], BF16, name=f"{label}_{d}_{gp}",
                            tag=f"{label}{d}_{gp}",
                        )
                        nc.sync.dma_start(
                            out=t_[:, :],
                            in_=dram[d * 128:(d + 1) * 128, gp * 256:(gp + 1) * 256],
                        )
                        for gh in range(2):
                            ts_[(d, 2 * gp + gh)] = t_[:, gh * 128:(gh + 1) * 128]
                return ts_

            def load_x_chunk(xdram, c, label):
                tiles = []
                for d in range(DT):
                    t_ = xpool.tile([128, CHUNK], BF16, name=f"{label}{c}_{d}", tag="xs")
                    nc.sync.dma_start(
                        out=t_[:, :],
                        in_=xdram[d * 128:(d + 1) * 128, c * CHUNK:(c + 1) * CHUNK],
                    )
                    tiles.append(t_)
                return tiles

            # DMA issue order tracks the prologue's consumption order so the
            # first matmuls start after ~1.3MB instead of after all weights.
            # In loop mode the body loads its own chunk-0 tiles instead
            # (steady-state: that load sits at the iteration boundary).
            ext_x0 = loop_n is None
            xk0 = load_x_chunk(xk, 0, "xk") if ext_x0 else None
            wk_t = load_w_tiles(wk, "wk")
            bk_t = cpool.tile([128, G], F32, name="bk_t", tag="bk")
            nc.sync.dma_start(out=bk_t[:, :], in_=bk[:, :])

            xq0 = load_x_chunk(xq, 0, "xq") if ext_x0 else None
            wq_t = load_w_tiles(wq, "wq")
            bq_t = cpool.tile([128, G], F32, name="bq_t", tag="bq")
            nc.sync.dma_start(out=bq_t[:, :], in_=bq[:, :])

            trim_t = cpool.tile([128, 128], F32, name="trim_t", tag="trim")
            nc.sync.dma_start(out=trim_t[:, :], in_=trim[:, :])
            padb_t = cpool.tile([128, NBLK], F32, name="padb_t", tag="padb")
            nc.sync.dma_start(out=padb_t[:, :], in_=padb[:, :])

            xv0 = load_x_chunk(xv, 0, "xv") if ext_x0 else None
            wv_t = []
            for d in range(DT):
                t_ = wpool.tile([128, J], BF16, name=f"wv_t{d}", tag=f"wv{d}")
                nc.sync.dma_start(out=t_[:, :], in_=wv[d * 128:(d + 1) * 128, :])
                wv_t.append(t_)
            bvb_t = cpool.tile([128, J], F32, name="bvb_t", tag="bvb")
            nc.sync.dma_start(out=bvb_t[:, :], in_=bvb[:, :])

            wo_t = []
            for g in range(G):
                t_ = wpool.tile([128, D_M], BF16, name=f"wo_t{g}", tag=f"wo{g}")
                nc.sync.dma_start(out=t_[:, :], in_=wo[g * 128:(g + 1) * 128, :])
                wo_t.append(t_)
            bob_t = cpool.tile([128, D_M], F32, name="bob_t", tag="bob")
            nc.sync.dma_start(out=bob_t[:, :], in_=bob[:, :])

            # Persistent activations (bf16).
            khT = []  # khT[g]: [128, T] — heads 2g (rows 0-63), 2g+1 (64-127)
            for g in range(G):
                khT.append(ppool.tile([128, T], BF16, name=f"khT{g}", tag=f"khT{g}"))
            vh = []   # vh[i]: [128, 520] — per head h: cols 65h..65h+63 = v, 65h+64 = 1
            for i in range(NBLK):
                vh.append(ppool.tile([128, 520], BF16, name=f"vh{i}", tag=f"vh{i}"))

            # ---- emitters ---------------------------------------------------
            def kproj_group(c, g, xs):
                ps = psum_p.tile([128, CHUNK], F32, name=f"psK{c}_{g}", tag="pp")
                for d in range(DT):
                    nc.tensor.matmul(
                        ps[:, :], r(wk_t[(d, g)][:, :]), r(xs[d][:, :]),
                        start=(d == 0), stop=(d == DT - 1),
                    )
                nc.vector.tensor_scalar_add(
                    khT[g][:, c * CHUNK:(c + 1) * CHUNK], ps[:, :], bk_t[:, g:g + 1],
                )

            def vproj_group(c, tl, xs):
                i = 4 * c + tl
                ps = psum_p.tile([128, J], F32, name=f"psV{i}", tag="pp")
                for d in range(DT):
                    nc.tensor.matmul(
                        ps[:, :],
                        r(xs[d][:, tl * 128:(tl + 1) * 128]),
                        r(wv_t[d][:, :]),
                        start=(d == 0), stop=(d == DT - 1),
                    )
                dst = vh[i][:, 0:J + HPC].rearrange("p (h e) -> p h e", e=65)
                nc.vector.tensor_add(
                    dst[:, :, 0:64],
                    ps[:, :].rearrange("p (h e) -> p h e", e=64),
                    bvb_t[:, :].rearrange("p (h e) -> p h e", e=64),
                )
                nc.vector.memset(dst[:, :, 64:65], 1.0)

            def qproj_group(c, g, xs):
                qt = qpool.tile([128, CHUNK], BF16, name=f"qht{c}_{g}", tag="qht")
                ps = psum_p.tile([128, CHUNK], F32, name=f"psQ{c}_{g}", tag="pp")
                for d in range(DT):
                    nc.tensor.matmul(
                        ps[:, :], r(wq_t[(d, g)][:, :]), r(xs[d][:, :]),
                        start=(d == 0), stop=(d == DT - 1),
                    )
                nc.vector.tensor_scalar_add(qt[:, :], ps[:, :], bq_t[:, g:g + 1])
                return qt

            def out_group(c, mt, e, ants):
                row0 = (4 * c + mt) * 128
                ps = psum_p.tile([128, CHUNK], F32, name=f"psO{c}_{mt}_{e}", tag="pp")
                for g in range(G):
                    nc.tensor.matmul(
                        ps[:, :],
                        r(ants[g][:, mt * 128:(mt + 1) * 128]),
                        r(wo_t[g][:, e * CHUNK:(e + 1) * CHUNK]),
                        start=(g == 0), stop=(g == G - 1),
                    )
                ob = opool.tile([128, CHUNK], F32, name=f"ob{c}_{mt}_{e}", tag="ob")
                nc.vector.tensor_add(ob[:, :], ps[:, :],
                                     bob_t[:, e * CHUNK:(e + 1) * CHUNK])
                nc.sync.dma_start(
                    out=out[row0:row0 + 128, e * CHUNK:(e + 1) * CHUNK],
                    in_=ob[:, :],
                )

            def attention_chunk(c, qht, fill):
                """Attention for chunk c; `fill` = list of closures emitting
                independent PE work, spread evenly between blocks.  No filler
                is placed in the last 2 blocks of each head-group so the
                normalize chain (recip/bcast/mul) isn't queued behind filler
                DVE ops — the next group's first A matmul waits on it."""
                nb = 4 * c + 4
                nfill = len(fill)
                fi = 0
                navail = G * (nb - 2)
                done = 0
                ants = []
                for g in range(G):
                    at = apool.tile([128, CHUNK], BF16, name=f"ant{c}_{g}", tag="ant")
                    ants.append(at)
                    ps_a = [psum_a.tile([65, CHUNK], F32,
                                        name=f"psA{c}_{2 * g + hh}", tag="pa")
                            for hh in range(2)]
                    # 3-deep software pipeline: A(bk-3) is emitted after S(bk),
                    # so each A pair waits on an exp that had ~2 blocks of
                    # ACT time to finish.
                    pend = []  # [(ms, es), ...] oldest first

                    def emit_a(bk_, first):
                        pcs, pes = pend.pop(0)
                        for hh in range(2):
                            nc.tensor.matmul(
                                ps_a[hh][:, pcs:],
                                r(vh[bk_][:, 65 * (2 * g + hh):65 * (2 * g + hh) + 65]),
                                r(pes[:, 512 * hh + pcs:512 * hh + 512]),
                                start=first, stop=(bk_ == nb - 1),
                            )

                    for bk in range(nb):
                        m = bk - 4 * c  # >=0 on the diagonal superblock
                        ms = 128 * m if m > 0 else 0
                        ps_s = psum_s.tile([128, 2 * CHUNK], F32,
                                           name=f"psS{c}_{g}_{bk}", tag="ps")
                        for hh in range(2):
                            nc.tensor.matmul(
                                ps_s[:, 512 * hh + ms:512 * hh + 512],
                                r(khT[g][hh * 64:(hh + 1) * 64, bk * 128:(bk + 1) * 128]),
                                r(qht[g][hh * 64:(hh + 1) * 64, ms:]),
                                start=True, stop=True,
                            )
                        if m >= 0:
                            # triangular additive mask on both heads' diag blocks
                            for hh in range(2):
                                nc.vector.tensor_add(
                                    ps_s[:, 512 * hh + ms:512 * hh + ms + 128],
                                    ps_s[:, 512 * hh + ms:512 * hh + ms + 128],
                                    trim_t[:, :],
                                )
                        if bk >= 3:
                            emit_a(bk - 3, first=(bk == 3))
                        es = epool.tile([128, 2 * CHUNK], BF16,
                                        name=f"es{c}_{g}_{bk}", tag="es")
                        if ms > 0:
                            nc.scalar.activation(
                                es[:, 0:2 * CHUNK].rearrange(
                                    "p (h e) -> p h e", h=2)[:, :, ms:],
                                ps_s[:, 0:2 * CHUNK].rearrange(
                                    "p (h e) -> p h e", h=2)[:, :, ms:],
                                Exp, bias=padb_t[:, bk:bk + 1], scale=0.125,
                            )
                        else:
                            nc.scalar.activation(
                                es[:, :], ps_s[:, :], Exp,
                                bias=padb_t[:, bk:bk + 1], scale=0.125,
                            )
                        pend.append((ms, es))
                        if bk < nb - 2:
                            done += 1
                            want = nfill * done // navail
                            while fi < want:
                                fill[fi]()
                                fi += 1
                    emit_a(nb - 3, first=False)
                    emit_a(nb - 2, first=False)
                    emit_a(nb - 1, first=False)
                    for hh in range(2):
                        h = 2 * g + hh
                        # normalize: rows 0-63 = A^T numerator, row 64 = denom.
                        # NB: partition_broadcast reads partition 0 of the
                        # underlying tile regardless of the input AP's
                        # partition offset, so the reciprocal must land on
                        # partition 0.
                        rc = rpool.tile([128, CHUNK], F32, name=f"rc{c}_{h}", tag="rc")
                        nc.vector.reciprocal(rc[0:1, :], ps_a[hh][64:65, :])
                        rb = rpool.tile([128, CHUNK], F32, name=f"rb{c}_{h}", tag="rb")
                        nc.gpsimd.partition_broadcast(rb[0:64, :], rc[0:1, :])
                        nc.vector.tensor_mul(
                            at[hh * 64:(hh + 1) * 64, :], ps_a[hh][0:64, :], rb[0:64, :],
                        )
                while fi < nfill:
                    fill[fi]()
                    fi += 1
                return ants

            def body(x0=None):
                # ---- prologue: chunk 0 projections --------------------------
                if x0 is None:
                    xk_0 = load_x_chunk(xk, 0, "xk")
                    xv_0 = load_x_chunk(xv, 0, "xv")
                    xq_0 = load_x_chunk(xq, 0, "xq")
                else:
                    xk_0, xv_0, xq_0 = x0
                for g in range(G):
                    kproj_group(0, g, xk_0)
                qht_cur = [qproj_group(0, g, xq_0) for g in range(G)]
                for tl in range(4):
                    vproj_group(0, tl, xv_0)

                # out(c) runs as filler two chunks later (att(c+2)) so that
                # chunk 3 — where the ACT(exp) deficit is largest — has 16
                # fill units (out(1) + out(2)) instead of 8.
                all_ants = [None] * NCHUNK
                for c in range(NCHUNK):
                    fill = []
                    outs_now = []
                    if c == 1:
                        outs_now = [0]
                    elif c == 3:
                        outs_now = [1, 2]
                    for co in outs_now:
                        for mt in range(4):
                            for e in range(2):
                                fill.append(lambda mt=mt, e=e, a=all_ants[co], cc=co:
                                            out_group(cc, mt, e, a))
                    if c + 1 < NCHUNK:
                        xkn = load_x_chunk(xk, c + 1, "xk")
                        xvn = load_x_chunk(xv, c + 1, "xv")
                        xqn = load_x_chunk(xq, c + 1, "xq")
                        for g in range(G):
                            fill.append(lambda g=g, xs=xkn, cc=c + 1:
                                        kproj_group(cc, g, xs))
                        for tl in range(4):
                            fill.append(lambda tl=tl, xs=xvn, cc=c + 1:
                                        vproj_group(cc, tl, xs))
                        qht_next = [None] * G

                        def mk_q(g, xs, cc):
                            def go():
                                qht_next[g] = qproj_group(cc, g, xs)
                            return go
                        for g in range(G):
                            fill.append(mk_q(g, xqn, c + 1))
                    else:
                        qht_next = None

                    all_ants[c] = attention_chunk(c, qht_cur, fill)
                    qht_cur = qht_next

                # tail: out projection of the last chunk
                for mt in range(4):
                    for e in range(2):
                        out_group(NCHUNK - 1, mt, e, all_ants[NCHUNK - 1])

            if loop_n is not None:
                with tc.For_i(0, loop_n, 1):
                    body()
            else:
                body((xk0, xv0, xq0))
                for _ in range(reps - 1):
                    body()

    nc.finalize()
    return nc


def get_nc():
    global _cached_nc
    if _cached_nc is None:
        _cached_nc = build_nc()
    return _cached_nc


def make_in_maps(q, k, v, pad_mask, Wq, bq, Wk, bk, Wv, bv, Wo, bo):
    """Host-side sharding: core c -> batch c//2, head-group c%2."""
    import ml_dtypes
    bf = ml_dtypes.bfloat16
    f = np.float32
    tri = np.where(
        np.arange(128)[None, :] >= np.arange(128)[:, None], 0.0, NEG
    ).astype(f)  # [tk, tq]: allow tq >= tk
    in_maps = []
    xT = {}
    for n in range(N_B):
        xT[n] = (
            np.ascontiguousarray(np.asarray(q[n], f).T.astype(bf)),
            np.ascontiguousarray(np.asarray(k[n], f).T.astype(bf)),
            np.ascontiguousarray(np.asarray(v[n], f).T.astype(bf)),
        )
    for c in range(N_CORES):
        n, grp = divmod(c, 2)
        js = slice(grp * J, (grp + 1) * J)
        pb = np.where(np.asarray(pad_mask[n]) == 0, NEG, 0.0).astype(f)
        in_maps.append({
            "xq_t": xT[n][0], "xk_t": xT[n][1], "xv_t": xT[n][2],
            "wq": np.ascontiguousarray(np.asarray(Wq, f)[:, js].astype(bf)),
            "wk": np.ascontiguousarray(np.asarray(Wk, f)[:, js].astype(bf)),
            "wv": np.ascontiguousarray(np.asarray(Wv, f)[:, js].astype(bf)),
            "bq2": np.ascontiguousarray(np.asarray(bq, f)[js].reshape(G, 128).T),
            "bk2": np.ascontiguousarray(np.asarray(bk, f)[js].reshape(G, 128).T),
            "bvb": np.broadcast_to(np.asarray(bv, f)[js], (128, J)).copy(),
            "wo": np.ascontiguousarray(np.asarray(Wo, f)[js, :].astype(bf)),
            "bob": (np.broadcast_to(np.asarray(bo, f), (128, D_M)).copy()
                    if grp == 0 else np.zeros((128, D_M), f)),
            "padb": np.ascontiguousarray(pb.reshape(NBLK, 128).T),
            "trimask": tri,
        })
    return in_maps


def kernel(**inputs) -> np.ndarray:
    nc = get_nc()
    in_maps = make_in_maps(**inputs)
    res = run_bass_kernel_spmd(nc, in_maps, list(range(N_CORES))).results
    out = np.empty((N_B, T, D_M), np.float32)
    for n in range(N_B):
        out[n] = res[2 * n]["out"] + res[2 * n + 1]["out"]
    return out


# revision 3
# speedup vs baseline: 1.2376x; 1.0165x over previous
"""Trainium2 Bass kernel for 16-head causal MultiHeadAttention (v2).

Problem: N=4, T=2048, D_M=1024, HEADS=16, D_K=64, fp32, causal + key pad mask.

Sharding (8 cores): core c handles batch n = c//2 and head group g = c%2
(8 heads each).  Each core computes its batch's projections restricted to its
8 heads, causal attention for those heads, and a partial output projection
(A_heads @ Wo_rows).  The host sums the two partials per batch.

v2 changes vs v1:
  - x / khT / vh / qht / es / at tiles in bf16 (inputs + attention operands).
    All matmul operands are bf16 (the compiler rejects mixed bf16/fp32r
    matmuls); PSUM accumulation stays fp32.  Measured end-to-end rel err
    ~5e-3, well under the 2e-2 gate.  Halves x DMA traffic and SBUF use, and
    lifts the fp32r free>=256 matmul constraint so causal diag blocks compute
    exactly the needed columns.
  - wq and wk live in separate SBUF slots (no mid-kernel weight reload).
  - Chunk-level software pipeline: projections of chunk c+1 and the output
    projection of chunk c-1 are emitted as PE "filler" interleaved between
    attention blocks of chunk c, so the Tensor engine never waits for the
    exp (ACT) stream.  PSUM budget: 2x[128,1024] score tiles + 2x[65,512]
    attn accumulators + 2x[128,512] projection tiles = 8 banks exactly.

Device dataflow (transpose-free), as v1:
  - Host passes q/k/v pre-transposed (xT: [D_M, T], bf16) so the projection
    contraction dim lands on SBUF partitions.
  - Scores computed transposed S^T[tk, tq]; softmax exp is elementwise from
    PSUM; attn@V consumes exp(S^T) directly; a ones-column in vh makes row 64
    of the attn@V accumulator the softmax denominator.
  - A_norm^T is the lhsT the output projection needs; out[tq, e] lands in
    natural layout.
"""

import os
import sys

import numpy as np

for _p in ("/opt/trn_rl_repo",):
    if _p not in sys.path and os.path.isdir(_p):
        sys.path.insert(0, _p)

import concourse.bacc as bacc
import concourse.bass as bass
import concourse.mybir as mybir
import concourse.tile as tile
from concourse.bass_utils import run_bass_kernel_spmd

# Problem constants (hardcoded per harness contract).
D_M = 1024
HEADS = 16
D_K = 64
N_B = 4
T = 2048
N_CORES = 8
HPC = HEADS // 2          # heads per core = 8
J = HPC * D_K             # per-core projection width = 512
G = J // 128              # j-tiles per core = 4
DT = D_M // 128           # d (contraction) tiles = 8
CHUNK = 512               # tq chunk (free dim of most matmuls)
NCHUNK = T // CHUNK       # 4
NBLK = T // 128           # tk blocks = 16
F32 = mybir.dt.float32
F32R = mybir.dt.float32r
BF16 = mybir.dt.bfloat16
NEG = -1.0e30

_cached_nc = None


def r(ap):
    """matmul-operand view: f32 tiles as float32r, bf16 passes through."""
    if ap.dtype in (F32R, BF16):
        return ap
    return ap.bitcast(F32R)


def build_nc(loop_n=None, reps=1):
    """loop_n: wrap the body in a HW For_i loop (used for loop-slope timing).
    reps: emit the body N times sequentially (sim-only steady-state probe)."""
    nc = bacc.Bacc(None)

    xq = nc.declare_dram_parameter("xq_t", [D_M, T], BF16, isOutput=False)
    xk = nc.declare_dram_parameter("xk_t", [D_M, T], BF16, isOutput=False)
    xv = nc.declare_dram_parameter("xv_t", [D_M, T], BF16, isOutput=False)
    wq = nc.declare_dram_parameter("wq", [D_M, J], BF16, isOutput=False)
    wk = nc.declare_dram_parameter("wk", [D_M, J], BF16, isOutput=False)
    wv = nc.declare_dram_parameter("wv", [D_M, J], BF16, isOutput=False)
    bq = nc.declare_dram_parameter("bq2", [128, G], F32, isOutput=False)
    bk = nc.declare_dram_parameter("bk2", [128, G], F32, isOutput=False)
    bvb = nc.declare_dram_parameter("bvb", [128, J], F32, isOutput=False)
    wo = nc.declare_dram_parameter("wo", [J, D_M], BF16, isOutput=False)
    bob = nc.declare_dram_parameter("bob", [128, D_M], F32, isOutput=False)
    padb = nc.declare_dram_parameter("padb", [128, NBLK], F32, isOutput=False)
    trim = nc.declare_dram_parameter("trimask", [128, 128], F32, isOutput=False)
    out = nc.declare_dram_parameter("out", [T, D_M], F32, isOutput=True)

    Exp = mybir.ActivationFunctionType.Exp

    with tile.TileContext(nc) as tc:
        with (
            tc.tile_pool(name="consts", bufs=1) as cpool,
            tc.tile_pool(name="wproj", bufs=1) as wpool,
            tc.tile_pool(name="persist", bufs=1) as ppool,
            tc.tile_pool(name="xs", bufs=32) as xpool,
            tc.tile_pool(name="qht", bufs=8) as qpool,
            tc.tile_pool(name="ant", bufs=12) as apool,
            tc.tile_pool(name="exps", bufs=4) as epool,
            tc.tile_pool(name="rec", bufs=2) as rpool,
            tc.tile_pool(name="osb", bufs=3) as opool,
            tc.tile_pool(name="ps_s", bufs=2, space="PSUM") as psum_s,
            tc.tile_pool(name="ps_a", bufs=2, space="PSUM") as psum_a,
            tc.tile_pool(name="ps_p", bufs=2, space="PSUM") as psum_p,
        ):
            # ---- persistent weight/const loads (DMA order = consumption order)
            def load_w_tiles(dram, label):
                """Load [128,128] weight tiles as [128,256] g-pairs: 512-byte
                contiguous runs keep the DMA at full rate (bf16 128-col tiles
                would halve it)."""
                ts_ = {}
                for gp in range(G // 2):
                    for d in range(DT):
                        t_ = wpool.tile(
                            [128, 256], BF16, name=f"{label}_{d}_{gp}",
                            tag=f"{label}{d}_{gp}",
                        )
                        nc.sync.dma_start(
                            out=t_[:, :],
                            in_=dram[d * 128:(d + 1) * 128, gp * 256:(gp + 1) * 256],
                        )
                        for gh in range(2):
                            ts_[(d, 2 * gp + gh)] = t_[:, gh * 128:(gh + 1) * 128]
                return ts_

            def load_x_chunk(xdram, c, label):
                tiles = []
                for d in range(DT):
                    t_ = xpool.tile([128, CHUNK], BF16, name=f"{label}{c}_{d}", tag="xs")
                    nc.sync.dma_start(
                        out=t_[:, :],
                        in_=xdram[d * 128:(d + 1) * 128, c * CHUNK:(c + 1) * CHUNK],
                    )
                    tiles.append(t_)
                return tiles

            # DMA issue order tracks the prologue's consumption order so the
            # first matmuls start after ~1.3MB instead of after all weights.
            # In loop mode the body loads its own chunk-0 tiles instead
            # (steady-state: that load sits at the iteration boundary).
            ext_x0 = loop_n is None
            xk0 = load_x_chunk(xk, 0, "xk") if ext_x0 else None
            wk_t = load_w_tiles(wk, "wk")
            bk_t = cpool.tile([128, G], F32, name="bk_t", tag="bk")
            nc.sync.dma_start(out=bk_t[:, :], in_=bk[:, :])

            xq0 = load_x_chunk(xq, 0, "xq") if ext_x0 else None
            wq_t = load_w_tiles(wq, "wq")
            bq_t = cpool.tile([128, G], F32, name="bq_t", tag="bq")
            nc.sync.dma_start(out=bq_t[:, :], in_=bq[:, :])

            trim_t = cpool.tile([128, 128], F32, name="trim_t", tag="trim")
            nc.sync.dma_start(out=trim_t[:, :], in_=trim[:, :])
            padb_t = cpool.tile([128, NBLK], F32, name="padb_t", tag="padb")
            nc.sync.dma_start(out=padb_t[:, :], in_=padb[:, :])

            xv0 = load_x_chunk(xv, 0, "xv") if ext_x0 else None
            wv_t = []
            for d in range(DT):
                t_ = wpool.tile([128, J], BF16, name=f"wv_t{d}", tag=f"wv{d}")
                nc.sync.dma_start(out=t_[:, :], in_=wv[d * 128:(d + 1) * 128, :])
                wv_t.append(t_)
            bvb_t = cpool.tile([128, J], F32, name="bvb_t", tag="bvb")
            nc.sync.dma_start(out=bvb_t[:, :], in_=bvb[:, :])

            wo_t = []
            for g in range(G):
                t_ = wpool.tile([128, D_M], BF16, name=f"wo_t{g}", tag=f"wo{g}")
                nc.sync.dma_start(out=t_[:, :], in_=wo[g * 128:(g + 1) * 128, :])
                wo_t.append(t_)
            bob_t = cpool.tile([128, D_M], F32, name="bob_t", tag="bob")
            nc.sync.dma_start(out=bob_t[:, :], in_=bob[:, :])

            # Persistent activations (bf16).
            khT = []  # khT[g]: [128, T] — heads 2g (rows 0-63), 2g+1 (64-127)
            for g in range(G):
                khT.append(ppool.tile([128, T], BF16, name=f"khT{g}", tag=f"khT{g}"))
            vh = []   # vh[i]: [128, 520] — per head h: cols 65h..65h+63 = v, 65h+64 = 1
            for i in range(NBLK):
                vh.append(ppool.tile([128, 520], BF16, name=f"vh{i}", tag=f"vh{i}"))

            # ---- emitters ---------------------------------------------------
            def kproj_group(c, g, xs):
                ps = psum_p.tile([128, CHUNK], F32, name=f"psK{c}_{g}", tag="pp")
                for d in range(DT):
                    nc.tensor.matmul(
                        ps[:, :], r(wk_t[(d, g)][:, :]), r(xs[d][:, :]),
                        start=(d == 0), stop=(d == DT - 1),
                    )
                nc.vector.tensor_scalar_add(
                    khT[g][:, c * CHUNK:(c + 1) * CHUNK], ps[:, :], bk_t[:, g:g + 1],
                )

            def vproj_group(c, tl, xs):
                i = 4 * c + tl
                ps = psum_p.tile([128, J], F32, name=f"psV{i}", tag="pp")
                for d in range(DT):
                    nc.tensor.matmul(
                        ps[:, :],
                        r(xs[d][:, tl * 128:(tl + 1) * 128]),
                        r(wv_t[d][:, :]),
                        start=(d == 0), stop=(d == DT - 1),
                    )
                dst = vh[i][:, 0:J + HPC].rearrange("p (h e) -> p h e", e=65)
                nc.vector.tensor_add(
                    dst[:, :, 0:64],
                    ps[:, :].rearrange("p (h e) -> p h e", e=64),
                    bvb_t[:, :].rearrange("p (h e) -> p h e", e=64),
                )
                nc.vector.memset(dst[:, :, 64:65], 1.0)

            def qproj_group(c, g, xs):
                qt = qpool.tile([128, CHUNK], BF16, name=f"qht{c}_{g}", tag="qht")
                ps = psum_p.tile([128, CHUNK], F32, name=f"psQ{c}_{g}", tag="pp")
                for d in range(DT):
                    nc.tensor.matmul(
                        ps[:, :], r(wq_t[(d, g)][:, :]), r(xs[d][:, :]),
                        start=(d == 0), stop=(d == DT - 1),
                    )
                nc.vector.tensor_scalar_add(qt[:, :], ps[:, :], bq_t[:, g:g + 1])
                return qt

            def out_group(c, mt, e, ants):
                row0 = (4 * c + mt) * 128
                ps = psum_p.tile([128, CHUNK], F32, name=f"psO{c}_{mt}_{e}", tag="pp")
                for g in range(G):
                    nc.tensor.matmul(
                        ps[:, :],
                        r(ants[g][:, mt * 128:(mt + 1) * 128]),
                        r(wo_t[g][:, e * CHUNK:(e + 1) * CHUNK]),
                        start=(g == 0), stop=(g == G - 1),
                    )
                ob = opool.tile([128, CHUNK], F32, name=f"ob{c}_{mt}_{e}", tag="ob")
                nc.vector.tensor_add(ob[:, :], ps[:, :],
                                     bob_t[:, e * CHUNK:(e + 1) * CHUNK])
                nc.sync.dma_start(
                    out=out[row0:row0 + 128, e * CHUNK:(e + 1) * CHUNK],
                    in_=ob[:, :],
                )

            def attention_chunk(c, qht, fill):
                """Attention for chunk c; `fill` = list of closures emitting
                independent PE work, spread evenly between blocks.  No filler
                is placed in the last 2 blocks of each head-group so the
                normalize chain (recip/bcast/mul) isn't queued behind filler
                DVE ops — the next group's first A matmul waits on it."""
                nb = 4 * c + 4
                nfill = len(fill)
                fi = 0
                navail = G * (nb - 2)
                done = 0
                ants = []
                for g in range(G):
                    at = apool.tile([128, CHUNK], BF16, name=f"ant{c}_{g}", tag="ant")
                    ants.append(at)
                    ps_a = [psum_a.tile([65, CHUNK], F32,
                                        name=f"psA{c}_{2 * g + hh}", tag="pa")
                            for hh in range(2)]
                    # 3-deep software pipeline: A(bk-3) is emitted after S(bk),
                    # so each A pair waits on an exp that had ~2 blocks of
                    # ACT time to finish.
                    pend = []  # [(ms, es), ...] oldest first

                    def emit_a(bk_, first):
                        pcs, pes = pend.pop(0)
                        for hh in range(2):
                            nc.tensor.matmul(
                                ps_a[hh][:, pcs:],
                                r(vh[bk_][:, 65 * (2 * g + hh):65 * (2 * g + hh) + 65]),
                                r(pes[:, 512 * hh + pcs:512 * hh + 512]),
                                start=first, stop=(bk_ == nb - 1),
                            )

                    for bk in range(nb):
                        m = bk - 4 * c  # >=0 on the diagonal superblock
                        ms = 128 * m if m > 0 else 0
                        ps_s = psum_s.tile([128, 2 * CHUNK], F32,
                                           name=f"psS{c}_{g}_{bk}", tag="ps")
                        for hh in range(2):
                            nc.tensor.matmul(
                                ps_s[:, 512 * hh + ms:512 * hh + 512],
                                r(khT[g][hh * 64:(hh + 1) * 64, bk * 128:(bk + 1) * 128]),
                                r(qht[g][hh * 64:(hh + 1) * 64, ms:]),
                                start=True, stop=True,
                            )
                        if m >= 0:
                            # triangular additive mask on both heads' diag blocks
                            for hh in range(2):
                                nc.vector.tensor_add(
                                    ps_s[:, 512 * hh + ms:512 * hh + ms + 128],
                                    ps_s[:, 512 * hh + ms:512 * hh + ms + 128],
                                    trim_t[:, :],
                                )
                        if bk >= 3:
                            emit_a(bk - 3, first=(bk == 3))
                        es = epool.tile([128, 2 * CHUNK], BF16,
                                        name=f"es{c}_{g}_{bk}", tag="es")
                        if ms > 0:
                            nc.scalar.activation(
                                es[:, 0:2 * CHUNK].rearrange(
                                    "p (h e) -> p h e", h=2)[:, :, ms:],
                                ps_s[:, 0:2 * CHUNK].rearrange(
                                    "p (h e) -> p h e", h=2)[:, :, ms:],
                                Exp, bias=padb_t[:, bk:bk + 1], scale=0.125,
                            )
                        else:
                            nc.scalar.activation(
                                es[:, :], ps_s[:, :], Exp,
                                bias=padb_t[:, bk:bk + 1], scale=0.125,
                            )
                        pend.append((ms, es))
                        if bk < nb - 2:
                            done += 1
                            want = nfill * done // navail
                            while fi < want:
                                fill[fi]()
                                fi += 1
                    emit_a(nb - 3, first=False)
                    emit_a(nb - 2, first=False)
                    emit_a(nb - 1, first=False)
                    for hh in range(2):
                        h = 2 * g + hh
                        # normalize: rows 0-63 = A^T numerator, row 64 = denom.
                        # NB: partition_broadcast reads partition 0 of the
                        # underlying tile regardless of the input AP's
                        # partition offset, so the reciprocal must land on
                        # partition 0.
                        rc = rpool.tile([128, CHUNK], F32, name=f"rc{c}_{h}", tag="rc")
                        nc.vector.reciprocal(rc[0:1, :], ps_a[hh][64:65, :])
                        rb = rpool.tile([128, CHUNK], F32, name=f"rb{c}_{h}", tag="rb")
                        nc.gpsimd.partition_broadcast(rb[0:64, :], rc[0:1, :])
                        nc.vector.tensor_mul(
                            at[hh * 64:(hh + 1) * 64, :], ps_a[hh][0:64, :], rb[0:64, :],
                        )
                while fi < nfill:
                    fill[fi]()
                    fi += 1
                return ants

            def body(x0=None):
                # ---- prologue: chunk 0 projections --------------------------
                if x0 is None:
                    xk_0 = load_x_chunk(xk, 0, "xk")
                    xv_0 = load_x_chunk(xv, 0, "xv")
                    xq_0 = load_x_chunk(xq, 0, "xq")
                else:
                    xk_0, xv_0, xq_0 = x0
                for g in range(G):
                    kproj_group(0, g, xk_0)
                qht_cur = [qproj_group(0, g, xq_0) for g in range(G)]
                for tl in range(4):
                    vproj_group(0, tl, xv_0)

                # out(c) runs as filler two chunks later (att(c+2)) so that
                # chunk 3 — where the ACT(exp) deficit is largest — has 16
                # fill units (out(1) + out(2)) instead of 8.
                all_ants = [None] * NCHUNK
                for c in range(NCHUNK):
                    fill = []
                    outs_now = []
                    if c == 1:
                        outs_now = [0]
                    elif c == 3:
                        outs_now = [1, 2]
                    for co in outs_now:
                        for mt in range(4):
                            for e in range(2):
                                fill.append(lambda mt=mt, e=e, a=all_ants[co], cc=co:
                                            out_group(cc, mt, e, a))
                    if c + 1 < NCHUNK:
                        xkn = load_x_chunk(xk, c + 1, "xk")
                        xvn = load_x_chunk(xv, c + 1, "xv")
                        xqn = load_x_chunk(xq, c + 1, "xq")
                        for g in range(G):
                            fill.append(lambda g=g, xs=xkn, cc=c + 1:
                                        kproj_group(cc, g, xs))
                        for tl in range(4):
                            fill.append(lambda tl=tl, xs=xvn, cc=c + 1:
                                        vproj_group(cc, tl, xs))
                        qht_next = [None] * G

                        def mk_q(g, xs, cc):
                            def go():
                                qht_next[g] = qproj_group(cc, g, xs)
                            return go
                        for g in range(G):
                            fill.append(mk_q(g, xqn, c + 1))
                    else:
                        qht_next = None

                    all_ants[c] = attention_chunk(c, qht_cur, fill)
                    qht_cur = qht_next

                # tail: out projection of the last chunk
                for mt in range(4):
                    for e in range(2):
                        out_group(NCHUNK - 1, mt, e, all_ants[NCHUNK - 1])

            if loop_n is not None:
                with tc.For_i(0, loop_n, 1):
                    for _ in range(reps):
                        body()
            else:
                body((xk0, xv0, xq0))
                for _ in range(reps - 1):
                    body()

    nc.finalize()
    return nc


def get_nc():
    global _cached_nc
    if _cached_nc is None:
        _cached_nc = build_nc()
    return _cached_nc


def make_in_maps(q, k, v, pad_mask, Wq, bq, Wk, bk, Wv, bv, Wo, bo):
    """Host-side sharding: core c -> batch c//2, head-group c%2."""
    import ml_dtypes
    bf = ml_dtypes.bfloat16
    f = np.float32
    tri = np.where(
        np.arange(128)[None, :] >= np.arange(128)[:, None], 0.0, NEG
    ).astype(f)  # [tk, tq]: allow tq >= tk
    in_maps = []
    xT = {}
    for n in range(N_B):
        xT[n] = (
            np.ascontiguousarray(np.asarray(q[n], f).T.astype(bf)),
            np.ascontiguousarray(np.asarray(k[n], f).T.astype(bf)),
            np.ascontiguousarray(np.asarray(v[n], f).T.astype(bf)),
        )
    for c in range(N_CORES):
        n, grp = divmod(c, 2)
        js = slice(grp * J, (grp + 1) * J)
        pb = np.where(np.asarray(pad_mask[n]) == 0, NEG, 0.0).astype(f)
        in_maps.append({
            "xq_t": xT[n][0], "xk_t": xT[n][1], "xv_t": xT[n][2],
            "wq": np.ascontiguousarray(np.asarray(Wq, f)[:, js].astype(bf)),
            "wk": np.ascontiguousarray(np.asarray(Wk, f)[:, js].astype(bf)),
            "wv": np.ascontiguousarray(np.asarray(Wv, f)[:, js].astype(bf)),
            "bq2": np.ascontiguousarray(np.asarray(bq, f)[js].reshape(G, 128).T),
            "bk2": np.ascontiguousarray(np.asarray(bk, f)[js].reshape(G, 128).T),
            "bvb": np.broadcast_to(np.asarray(bv, f)[js], (128, J)).copy(),
            "wo": np.ascontiguousarray(np.asarray(Wo, f)[js, :].astype(bf)),
            "bob": (np.broadcast_to(np.asarray(bo, f), (128, D_M)).copy()
                    if grp == 0 else np.zeros((128, D_M), f)),
            "padb": np.ascontiguousarray(pb.reshape(NBLK, 128).T),
            "trimask": tri,
        })
    return in_maps


def kernel(**inputs) -> np.ndarray:
    nc = get_nc()
    in_maps = make_in_maps(**inputs)
    res = run_bass_kernel_spmd(nc, in_maps, list(range(N_CORES))).results
    out = np.empty((N_B, T, D_M), np.float32)
    for n in range(N_B):
        out[n] = res[2 * n]["out"] + res[2 * n + 1]["out"]
    return out
